# revision 3
# baseline (speedup 1.0000x reference)
"""Trainium2 Bass kernel for BaseSpectrogram1D.

x[128, 131072] -> |DFT(window * overlapping_frames(x - mean))| [128, 511, 257]

Sharding: pure data parallel, batch dim split across 8 NeuronCores
(16 rows each). window/fourier_matrix are combined host-side into one
real [512, 512] fp16 matrix (Re bins 0..256 | Im bins 1..255; Im[0] and
Im[256] are exactly zero and omitted so a frame-tile's whole DFT fits a
single 512-wide PSUM bank).

The fp16 datapath unlocks the DMA xbar transpose (2-byte dtypes only):
x is cast to fp16 host-side and each batch row is transposed
DRAM->SBUF in one DMA, giving the [sample-in-chunk, chunk] layout the
TensorE contraction needs with zero PE/ACT transpose work. fp16
matmuls also get fast weight loads (FWL), unlike fp32/fp32r whose
in-matmul 4-byte weight load costs as much as the matmul itself.

Per core and batch row:
  - DMA-transpose x_f16[b] [1024,128] -> xt_raw [128, 1024]
  - DVE row-sum + ones-matmul -> mean on every partition, ACT scales
  - DVE tensor_scalar_add subtracts the mean -> xt fp16
  - per 128-frame tile: 4 accumulated fp16 matmuls (stride-2 column
    slices of xt are exactly frames^T) -> PSUM [128, 512]
  - DVE evicts PSUM -> fp16, GpSimd squares + pairs re^2+im^2,
    ACT Sqrt (+ |Re| for bins 0/256 straight from PSUM) -> DMA out
"""

import sys

if "/opt/trn_rl_repo" not in sys.path:
    sys.path.insert(0, "/opt/trn_rl_repo")

import numpy as np

L = 131072
B = 128
N = 512  # frame length
M = 511  # frames
STRIDE = 256
KH = 257  # one-sided bins
NCORES = 8
BPC = B // NCORES  # batches per core = 16

_CACHE = {}


def _tukey(n_pts, alpha=0.25):
    n = np.arange(n_pts, dtype=np.float64)
    edge = alpha * (n_pts - 1) / 2.0
    w = np.ones(n_pts)
    left = n < edge
    w[left] = 0.5 * (1.0 + np.cos(np.pi * (2.0 * n[left] / (alpha * (n_pts - 1)) - 1.0)))
    right = n > (n_pts - 1) - edge
    w[right] = 0.5 * (
        1.0 + np.cos(np.pi * (2.0 * n[right] / (alpha * (n_pts - 1)) - 2.0 / alpha + 1.0))
    )
    return w


def _default_consts():
    w = _tukey(N, 0.25)
    w = (w / w.sum()).astype(np.float32)
    nk = np.outer(np.arange(N, dtype=np.float64), np.arange(N, dtype=np.float64))
    sigma = np.exp(-2j * np.pi / N)
    fm = (sigma**nk)[:, :KH] * np.sqrt(N)
    return w, fm.astype(np.complex64)


def _build():
    """Build + schedule the Bass module once per process."""
    if "nc" in _CACHE:
        return _CACHE["nc"]

    import concourse.mybir as mybir
    import concourse.tile as tile
    from concourse import bacc

    F32 = mybir.dt.float32
    F16 = mybir.dt.float16
    AF = mybir.ActivationFunctionType
    ALU = mybir.AluOpType

    nc = bacc.Bacc(trn_type="TRN2", target_bir_lowering=False, debug=False)

    x_d = nc.dram_tensor("x", [BPC, 1024, 128], F16, kind="ExternalInput").ap()
    wfm_d = nc.dram_tensor("wfm", [4, 128, N], F16, kind="ExternalInput").ap()
    out_d = nc.dram_tensor("out", [BPC, M, KH], F32, kind="ExternalOutput").ap()

    with tile.TileContext(nc) as tc:
        with (
            tc.tile_pool(name="consts", bufs=1) as consts,
            tc.tile_pool(name="xt", bufs=3) as xtp,
            tc.tile_pool(name="small", bufs=4) as smallp,
            tc.tile_pool(name="sq", bufs=4) as sqp,
            tc.tile_pool(name="mag", bufs=4) as magp,
            tc.tile_pool(name="pmu", bufs=2, space="PSUM") as pmup,
            tc.tile_pool(name="pspec", bufs=3, space="PSUM") as pspecp,
        ):
            wfm_s = consts.tile([128, 4, N], F16)
            ones = consts.tile([128, 128], F32)
            nc.sync.dma_start(out=wfm_s, in_=wfm_d.rearrange("j p n -> p j n"))
            nc.vector.memset(ones, 1.0)

            for b in range(BPC):
                # one xbar transpose: xt_raw[e, c] = x16[b, c, e]
                xt_raw = xtp.tile([128, 1024], F16, tag="xtr")
                nc.sync.dma_start_transpose(out=xt_raw, in_=x_d[b])

                # batch mean -> all partitions
                part = smallp.tile([128, 1], F32)
                nc.vector.reduce_sum(part, xt_raw, axis=mybir.AxisListType.X)
                mu_ps = pmup.tile([128, 1], F32)
                nc.tensor.matmul(mu_ps, ones, part, start=True, stop=True)
                negmu = smallp.tile([128, 1], F32)
                nc.scalar.activation(negmu, mu_ps, AF.Copy, scale=-1.0 / L)

                xt = xtp.tile([128, 1024], F16, tag="xt")
                nc.vector.tensor_scalar_add(xt, xt_raw, negmu)
                xt3 = xt.rearrange("p (c two) -> p c two", two=2)

                # frame-tile matmuls + magnitude epilogue
                for mt in range(4):
                    m0 = mt * 128
                    mm = min(128, M - m0)
                    spec = pspecp.tile([128, N], F32)
                    for j in range(4):
                        lhsT = xt3[:, m0 + j // 2 : m0 + j // 2 + mm, j % 2]
                        nc.tensor.matmul(
                            spec[:mm],
                            lhsT,
                            wfm_s[:, j],
                            start=(j == 0),
                            stop=(j == 3),
                        )
                    # evict PSUM once on DVE (fp16), square+pair on GpSimd,
                    # sqrt on ACT; bins 0/256 are |Re| taken from PSUM
                    sp16 = sqp.tile([128, N], F16, tag="sp16")
                    nc.vector.tensor_copy(sp16[:mm], spec[:mm])
                    sq = sqp.tile([128, N], F16, tag="sq")
                    nc.gpsimd.tensor_mul(sq[:mm], sp16[:mm], sp16[:mm])
                    magsq = magp.tile([128, 256], F16, tag="magsq")
                    nc.gpsimd.tensor_add(
                        magsq[:mm], sq[:mm, 0:256], sq[:mm, 256:512]
                    )
                    mag = magp.tile([128, KH], F32, tag="mag")
                    nc.scalar.activation(
                        mag[:mm, 1:256], magsq[:mm, 1:256], AF.Sqrt
                    )
                    nc.scalar.activation(
                        mag[:mm, 0:257:256], spec[:mm, 0:257:256], AF.Abs
                    )
                    nc.sync.dma_start(out=out_d[b, m0 : m0 + mm, :], in_=mag[:mm])

    nc.compile()
    _CACHE["nc"] = nc
    return nc


def make_inputs(x, window=None, fourier_matrix=None):
    """Host-side prep: fp16 cast/layout + combined DFT matrix."""
    x = np.asarray(x, dtype=np.float32)
    if window is None or fourier_matrix is None:
        window, fourier_matrix = _default_consts()
    window = np.asarray(window)
    fourier_matrix = np.asarray(fourier_matrix)

    wfm = fourier_matrix.astype(np.complex64) * window.astype(np.float32)[:, None]
    wfm_cat = np.concatenate(
        [wfm.real[:, 0:257], wfm.imag[:, 1:256]], axis=1
    ).astype(np.float16)  # [512, 512]
    wfm_in = np.ascontiguousarray(wfm_cat.reshape(4, 128, N))

    x16 = np.ascontiguousarray(x.astype(np.float16).reshape(B, 1024, 128))
    return x16, wfm_in


def kernel(x, window=None, fourier_matrix=None, **_unused):
    from concourse.bass_utils import run_bass_kernel_spmd

    x16, wfm_in = make_inputs(x, window, fourier_matrix)
    nc = _build()
    in_maps = [
        {"x": x16[i * BPC : (i + 1) * BPC], "wfm": wfm_in} for i in range(NCORES)
    ]
    res = run_bass_kernel_spmd(nc, in_maps, core_ids=list(range(NCORES)))
    return np.concatenate([r["out"] for r in res.results], axis=0)


if __name__ == "__main__":
    rng = np.random.default_rng(0)
    x = rng.standard_normal((B, L)).astype(np.float32)
    out = kernel(x)
    print("out", out.shape, out.dtype, float(out.max()))


# revision 6
# speedup vs baseline: 1.1035x; 1.1035x over previous
"""Trainium2 Bass kernel for BaseSpectrogram1D.

x[128, 131072] -> |DFT(window * overlapping_frames(x - mean))| [128, 511, 257]

Sharding: pure data parallel, batch dim split across 8 NeuronCores
(16 rows each). window/fourier_matrix are combined host-side into one
real [512, 512] fp16 matrix (Re bins 0..256 | Im bins 1..255; Im[0] and
Im[256] are exactly zero and omitted so a frame-tile's whole DFT fits a
single 512-wide PSUM bank).

The fp16 datapath unlocks the DMA xbar transpose (2-byte dtypes only):
x is cast to fp16 host-side and each batch row is transposed
DRAM->SBUF in one DMA, giving the [sample-in-chunk, chunk] layout the
TensorE contraction needs with zero PE/ACT transpose work. fp16
matmuls also get fast weight loads (FWL), unlike fp32/fp32r whose
in-matmul 4-byte weight load costs as much as the matmul itself.

Structure per core:
  phase 0: all 16 xbar transposes + DVE row-sums, one batched
    ones-matmul + one ACT scale -> neg-means for every batch. Keeps
    the phase-1 PE stream dense so HAM stays at full clock.
  phase 1, per batch: DVE mean-subtract (fp16), then per 128-frame
    tile 4 accumulated fp16 matmuls (stride-2 column slices of the
    transposed x are exactly frames^T) -> PSUM [128, 512]. The last
    tile is padded to 128 frames (127-wide weights would disable FWL).
    Epilogue: squares from PSUM (ACT Square on even tiles, DVE
    copy+mul on odd tiles, balancing the two engines), GpSimd pairs
    re^2+im^2 (+ copies bins 0/256), ACT Sqrt emits |spec|, DMA out.
"""

import sys

if "/opt/trn_rl_repo" not in sys.path:
    sys.path.insert(0, "/opt/trn_rl_repo")

import numpy as np

L = 131072
B = 128
N = 512  # frame length
M = 511  # frames
STRIDE = 256
KH = 257  # one-sided bins
NCORES = 8
BPC = B // NCORES  # batches per core = 16

_CACHE = {}


def _tukey(n_pts, alpha=0.25):
    n = np.arange(n_pts, dtype=np.float64)
    edge = alpha * (n_pts - 1) / 2.0
    w = np.ones(n_pts)
    left = n < edge
    w[left] = 0.5 * (1.0 + np.cos(np.pi * (2.0 * n[left] / (alpha * (n_pts - 1)) - 1.0)))
    right = n > (n_pts - 1) - edge
    w[right] = 0.5 * (
        1.0 + np.cos(np.pi * (2.0 * n[right] / (alpha * (n_pts - 1)) - 2.0 / alpha + 1.0))
    )
    return w


def _default_consts():
    w = _tukey(N, 0.25)
    w = (w / w.sum()).astype(np.float32)
    nk = np.outer(np.arange(N, dtype=np.float64), np.arange(N, dtype=np.float64))
    sigma = np.exp(-2j * np.pi / N)
    fm = (sigma**nk)[:, :KH] * np.sqrt(N)
    return w, fm.astype(np.complex64)


def _build():
    """Build + schedule the Bass module once per process."""
    if "nc" in _CACHE:
        return _CACHE["nc"]

    import concourse.mybir as mybir
    import concourse.tile as tile
    from concourse import bacc

    F32 = mybir.dt.float32
    F16 = mybir.dt.float16
    AF = mybir.ActivationFunctionType

    nc = bacc.Bacc(trn_type="TRN2", target_bir_lowering=False, debug=False)

    x_d = nc.dram_tensor("x", [BPC, 1024, 128], F16, kind="ExternalInput").ap()
    wfm_d = nc.dram_tensor("wfm", [4, 128, N], F16, kind="ExternalInput").ap()
    out_d = nc.dram_tensor("out", [BPC, M, KH], F32, kind="ExternalOutput").ap()

    with tile.TileContext(nc) as tc:
        with (
            tc.tile_pool(name="consts", bufs=1) as consts,
            tc.tile_pool(name="xtraw", bufs=1) as xtrawp,
            tc.tile_pool(name="xt", bufs=3) as xtp,
            tc.tile_pool(name="sq", bufs=4) as sqp,
            tc.tile_pool(name="mag", bufs=4) as magp,
            tc.tile_pool(name="pmu", bufs=1, space="PSUM") as pmup,
            tc.tile_pool(name="pspec", bufs=4, space="PSUM") as pspecp,
        ):
            wfm_s = consts.tile([128, 4, N], F16)
            ones = consts.tile([128, 128], F32)
            parts = consts.tile([128, BPC], F32)
            negmu = consts.tile([128, BPC], F32)
            nc.sync.dma_start(out=wfm_s, in_=wfm_d.rearrange("j p n -> p j n"))
            nc.vector.memset(ones, 1.0)

            # phase 0: transposes + row sums + batched mean
            xts = []
            for b in range(BPC):
                xt_raw = xtrawp.tile([128, 1024], F16, tag=f"xtr{b}")
                nc.sync.dma_start_transpose(out=xt_raw, in_=x_d[b])
                nc.vector.reduce_sum(
                    parts[:, b : b + 1], xt_raw, axis=mybir.AxisListType.X
                )
                xts.append(xt_raw)
            mu_ps = pmup.tile([128, BPC], F32)
            nc.tensor.matmul(mu_ps, ones, parts, start=True, stop=True)
            nc.scalar.activation(negmu, mu_ps, AF.Copy, scale=-1.0 / L)

            # phase 1
            for b in range(BPC):
                xt = xtp.tile([128, 1028], F16, tag="xt")
                nc.vector.memset(xt[:, 1024:1028], 0.0)
                nc.vector.tensor_scalar_add(
                    xt[:, 0:1024], xts[b], negmu[:, b : b + 1]
                )
                xt3 = xt.rearrange("p (c two) -> p c two", two=2)

                for mt in range(4):
                    m0 = mt * 128
                    mm = min(128, M - m0)
                    spec = pspecp.tile([128, N], F32)
                    for j in range(4):
                        # always 128 output rows (frame 511 is junk, not stored)
                        lhsT = xt3[:, m0 + j // 2 : m0 + j // 2 + 128, j % 2]
                        nc.tensor.matmul(
                            spec,
                            lhsT,
                            wfm_s[:, j],
                            start=(j == 0),
                            stop=(j == 3),
                        )
                    # squares: alternate ACT / DVE to balance engines
                    sq = sqp.tile([128, N], F16, tag="sq")
                    if mt % 2 == 0:
                        nc.scalar.activation(sq[:mm], spec[:mm], AF.Square)
                    else:
                        sp16 = sqp.tile([128, N], F16, tag="sp16")
                        nc.vector.tensor_copy(sp16[:mm], spec[:mm])
                        nc.vector.tensor_mul(sq[:mm], sp16[:mm], sp16[:mm])
                    magsq = magp.tile([128, KH], F16, tag="magsq")
                    nc.gpsimd.tensor_add(
                        magsq[:mm, 1:256], sq[:mm, 1:256], sq[:mm, 257:512]
                    )
                    nc.gpsimd.tensor_copy(
                        magsq[:mm, 0:257:256], sq[:mm, 0:257:256]
                    )
                    mag = magp.tile([128, KH], F32, tag="mag")
                    nc.scalar.activation(mag[:mm], magsq[:mm], AF.Sqrt)
                    nc.sync.dma_start(out=out_d[b, m0 : m0 + mm, :], in_=mag[:mm])

    nc.compile()
    _CACHE["nc"] = nc
    return nc


def make_inputs(x, window=None, fourier_matrix=None):
    """Host-side prep: fp16 cast/layout + combined DFT matrix."""
    x = np.asarray(x, dtype=np.float32)
    if window is None or fourier_matrix is None:
        window, fourier_matrix = _default_consts()
    window = np.asarray(window)
    fourier_matrix = np.asarray(fourier_matrix)

    wfm = fourier_matrix.astype(np.complex64) * window.astype(np.float32)[:, None]
    wfm_cat = np.concatenate(
        [wfm.real[:, 0:257], wfm.imag[:, 1:256]], axis=1
    ).astype(np.float16)  # [512, 512]
    wfm_in = np.ascontiguousarray(wfm_cat.reshape(4, 128, N))

    x16 = np.ascontiguousarray(x.astype(np.float16).reshape(B, 1024, 128))
    return x16, wfm_in


def kernel(x, window=None, fourier_matrix=None, **_unused):
    from concourse.bass_utils import run_bass_kernel_spmd

    x16, wfm_in = make_inputs(x, window, fourier_matrix)
    nc = _build()
    in_maps = [
        {"x": x16[i * BPC : (i + 1) * BPC], "wfm": wfm_in} for i in range(NCORES)
    ]
    res = run_bass_kernel_spmd(nc, in_maps, core_ids=list(range(NCORES)))
    return np.concatenate([r["out"] for r in res.results], axis=0)


if __name__ == "__main__":
    rng = np.random.default_rng(0)
    x = rng.standard_normal((B, L)).astype(np.float32)
    out = kernel(x)
    print("out", out.shape, out.dtype, float(out.max()))
